# revision 12
# baseline (speedup 1.0000x reference)
"""Multi-head attention Bass/Tile kernel for Trainium2, 8-core SPMD.

Problem: B=2, S=2048, D=1024, H=16, DK=64 MHA forward returning
(out [B,S,D], attn [B,H,S,S]).

Sharding: core c handles batch b = c // 4 and head group g = c % 4
(heads 4g..4g+3).  Each core computes its 4 heads' attention entirely
locally; no collectives.  Host slices inputs / reassembles outputs.

Per-core pipeline (f32 data; matmul operands stored as float32r, which the
PE streams at full rate vs 1/4 rate for plain fp32):
  - transpose q,k,v tiles on PE (contraction dim must sit on partitions)
  - qhT,khT = W @ xT   (feature-major [256, S]);  vh = x @ WvT (token-major)
    with an appended ones-column -> AV matmul also yields softmax row-sums
  - per (head, 512-query chunk):
      scoresT[n,m] = khT.T-stationary @ qhT-moving   (PSUM)
      expT = exp(scoresT / 8)                        (ScalarE, PSUM->SBUF)
      outT' += vh'[ntile].T @ expT                   (PE accumulate, 65 rows)
      attn rows = transpose(expT) * recip(sums)      (PE transpose + VectorE)
"""

import numpy as np

import concourse.bass as bass
import concourse.mybir as mybir
from concourse import bacc
from concourse.tile import TileContext
from concourse.masks import make_identity

F32 = mybir.dt.float32
F32R = mybir.dt.float32r
F16 = mybir.dt.float16

B, S, D, H, DK = 2, 2048, 1024, 16, 64
P = 128
HPC = 4            # heads per core
HD = HPC * DK      # 256 head-dim columns per core
N_CORES = 8


def build_mha_kernel(seq=S, d_model=D, hpc=HPC, m_chunk=512, use_f32r=True):
    """Build the per-core Bass program. Returns the compiled Bacc object."""
    dk = DK
    hd = hpc * dk
    kt_n = d_model // P          # k-tiles over model dim (8)
    half = min(256, seq)         # prologue half-chunk (tokens)
    n_half = seq // half
    nch = seq // m_chunk         # query chunks per head
    ms_n = m_chunk // P          # 128-row subtiles per chunk
    nt_n = seq // P              # key tiles (16)
    dt_n = hd // P               # feature tiles of qhT/khT (2)

    AF = F32R if use_f32r else F32   # dtype of matmul operand tiles

    nc = bacc.Bacc("TRN2", target_bir_lowering=False, debug=False,
                   num_devices=N_CORES)

    q_d = nc.dram_tensor("q", [seq, d_model], F32, kind="ExternalInput").ap()
    k_d = nc.dram_tensor("k", [seq, d_model], F32, kind="ExternalInput").ap()
    v_d = nc.dram_tensor("v", [seq, d_model], F32, kind="ExternalInput").ap()
    wq_d = nc.dram_tensor("wq", [hd, d_model], F32, kind="ExternalInput").ap()
    wk_d = nc.dram_tensor("wk", [hd, d_model], F32, kind="ExternalInput").ap()
    wv_d = nc.dram_tensor("wv", [hd, d_model], F32, kind="ExternalInput").ap()
    attn_d = nc.dram_tensor("attn", [hpc, seq, seq], F32,
                            kind="ExternalOutput").ap()
    out_d = nc.dram_tensor("out", [seq, hd], F32, kind="ExternalOutput").ap()

    with TileContext(nc) as tc:
        with (
            tc.tile_pool(name="const", bufs=1) as const_pool,
            tc.tile_pool(name="wts", bufs=1) as wts_pool,
            tc.tile_pool(name="persist", bufs=1) as persist_pool,
            tc.tile_pool(name="xin", bufs=2) as xin_pool,
            tc.tile_pool(name="xt", bufs=2) as xt_pool,
            tc.tile_pool(name="expt", bufs=2 * nt_n + 2) as expt_pool,
            tc.tile_pool(name="arow", bufs=2) as arow_pool,
            tc.tile_pool(name="small", bufs=4) as small_pool,
            tc.tile_pool(name="outsb", bufs=2) as outsb_pool,
            tc.tile_pool(name="ps_a", bufs=2, space="PSUM") as ps_a,      # scores / proj
            tc.tile_pool(name="ps_t", bufs=2, space="PSUM") as ps_t,      # transposes
            tc.tile_pool(name="ps_av", bufs=2, space="PSUM") as ps_av,    # AV accum
            tc.tile_pool(name="ps_s", bufs=2, space="PSUM") as ps_s,      # small
        ):
            ident = const_pool.tile([P, P], F32)
            make_identity(nc, ident)
            ident_h = const_pool.tile([P, P], F16)
            nc.vector.tensor_copy(ident_h, ident)

            # ---- weights: load [hd, D] then transpose to wT[128k, kt, hd] ----
            wT = {}
            for name, w_dram in (("wq", wq_d), ("wk", wk_d), ("wv", wv_d)):
                w_sb = xin_pool.tile([P, dt_n, d_model], F32, tag="xload")
                nc.sync.dma_start(
                    out=w_sb,
                    in_=w_dram.rearrange("(a p) d -> p a d", p=P))
                wt_sb = wts_pool.tile([P, kt_n, hd], AF, tag=f"wt_{name}")
                for kt in range(kt_n):
                    pt = ps_t.tile([P, hd], F32, tag="t")
                    for a in range(dt_n):
                        nc.tensor.matmul(pt[:, a * P:(a + 1) * P],
                                         w_sb[:, a, kt * P:(kt + 1) * P],
                                         ident, is_transpose=True)
                    nc.vector.tensor_copy(wt_sb[:, kt, :], pt)
                wT[name] = wt_sb

            # ---- persistent activations ----
            qhT = persist_pool.tile([P, dt_n, seq], AF)   # [d-col, m]
            khT = persist_pool.tile([P, dt_n, seq], AF)
            vh = persist_pool.tile([P, nt_n, hpc, dk + 1], F16)  # + ones col
            ones_sb = const_pool.tile([P, nt_n], F32)
            nc.vector.memset(ones_sb, 1.0)
            for hh in range(hpc):  # ones column (f32 -> f32r rounding copy)
                nc.vector.tensor_copy(vh[:, :, hh, dk], ones_sb)

            # ---- transpose inputs + projections, streamed per half-chunk ----
            for name, x_dram in (("q", q_d), ("k", k_d), ("v", v_d)):
                for hc in range(n_half):
                    r0 = hc * half
                    sub_n = half // P  # 128-token subtiles in this half (2)
                    x_sb = xin_pool.tile([P, sub_n, d_model], F32, tag="xload")
                    nc.sync.dma_start(
                        out=x_sb,
                        in_=x_dram[r0:r0 + half, :].rearrange(
                            "(a p) d -> p a d", p=P))
                    xT = xt_pool.tile([P, kt_n, half], AF)
                    for kt in range(kt_n):
                        pt = ps_t.tile([P, half], F32, tag="t")
                        for a in range(sub_n):
                            nc.tensor.matmul(pt[:, a * P:(a + 1) * P],
                                             x_sb[:, a, kt * P:(kt + 1) * P],
                                             ident, is_transpose=True)
                        nc.scalar.copy(xT[:, kt, :], pt)  # ACT idle in prologue
                    if name in ("q", "k"):
                        dst = qhT if name == "q" else khT
                        wt_sb = wT["wq" if name == "q" else "wk"]
                        for dt in range(dt_n):
                            pp = ps_a.tile([P, half], F32, tag="a")
                            for kt in range(kt_n):
                                nc.tensor.matmul(
                                    pp,
                                    wt_sb[:, kt, dt * P:(dt + 1) * P],
                                    xT[:, kt, :],
                                    start=(kt == 0), stop=(kt == kt_n - 1))
                            nc.vector.tensor_copy(dst[:, dt, r0:r0 + half], pp)
                    else:
                        wt_sb = wT["wv"]
                        for a in range(sub_n):
                            nt = (r0 + a * P) // P
                            pp = ps_a.tile([P, hd], F32, tag="a")
                            for kt in range(kt_n):
                                nc.tensor.matmul(
                                    pp,
                                    xT[:, kt, a * P:(a + 1) * P],
                                    wt_sb[:, kt, :],
                                    start=(kt == 0), stop=(kt == kt_n - 1))
                            nc.vector.tensor_copy(
                                vh[:, nt, :, 0:dk],
                                pp.rearrange("p (h e) -> p h e", h=hpc))

            # ---- main loop: chunk outer, head inner ----
            scale = 1.0 / np.sqrt(np.float32(dk))
            for c in range(nch):
                m0 = c * m_chunk
                out_c = outsb_pool.tile([P, ms_n, hd], F32)
                for h in range(hpc):
                    dt_h, off_h = divmod(h * dk, P)

                    # scores + exp per key tile
                    expts = []
                    for nt in range(nt_n):
                        pscore = ps_a.tile([P, m_chunk], F32, tag="a")
                        nc.tensor.matmul(
                            pscore,
                            khT[off_h:off_h + dk, dt_h, nt * P:(nt + 1) * P],
                            qhT[off_h:off_h + dk, dt_h, m0:m0 + m_chunk])
                        et = expt_pool.tile([P, m_chunk], F16, tag="expt")
                        nc.scalar.activation(et, pscore,
                                             mybir.ActivationFunctionType.Exp,
                                             scale=float(scale))
                        expts.append(et)

                    # AV accumulation (row 64 = softmax denominator)
                    pav = ps_av.tile([dk + 1, m_chunk], F32, tag="av")
                    for nt in range(nt_n):
                        nc.tensor.matmul(
                            pav,
                            vh[:, nt, h, :],
                            expts[nt],
                            start=(nt == 0), stop=(nt == nt_n - 1))
                    av_sb = small_pool.tile([dk + 1, m_chunk], F32, tag="avsb")
                    nc.vector.tensor_copy(av_sb, pav)

                    for ms in range(ms_n):
                        # transpose [65, 128] -> [128, 65]: cols 0..63 are the
                        # unnormalized output rows, col 64 the softmax denom
                        po = ps_s.tile([P, dk + 1], F32, tag="s")
                        nc.tensor.matmul(po,
                                         av_sb[:, ms * P:(ms + 1) * P],
                                         ident[0:dk + 1, 0:dk + 1],
                                         is_transpose=True)
                        rcol = small_pool.tile([P, 1], F32, tag="rcolsb")
                        nc.vector.reciprocal(rcol, po[:, dk:dk + 1])

                        # attn row block [128 queries, seq keys]
                        arow = arow_pool.tile([P, seq], F32, tag="arow")
                        g4 = min(4, nt_n)
                        for ng in range(nt_n // g4):
                            pt = ps_t.tile([P, g4 * P], F16, tag="t")
                            for j in range(g4):
                                nt = ng * g4 + j
                                nc.tensor.matmul(
                                    pt[:, j * P:(j + 1) * P],
                                    expts[nt][:, ms * P:(ms + 1) * P],
                                    ident_h[:, 0:P], is_transpose=True)
                            nc.vector.tensor_scalar_mul(
                                arow[:, ng * g4 * P:(ng + 1) * g4 * P],
                                pt, rcol)
                        nc.sync.dma_start(
                            out=attn_d[h, m0 + ms * P:m0 + (ms + 1) * P, :],
                            in_=arow)

                        # output block [128 queries, dk]
                        nc.vector.tensor_scalar_mul(
                            out_c[:, ms, h * dk:(h + 1) * dk],
                            po[:, 0:dk], rcol)

                nc.sync.dma_start(
                    out=out_d[m0:m0 + m_chunk, :].rearrange(
                        "(a p) e -> p a e", p=P),
                    in_=out_c)

    nc.compile()
    return nc


_NC_CACHE = {}


def _get_compiled(key, **kw):
    if key not in _NC_CACHE:
        _NC_CACHE[key] = build_mha_kernel(**kw)
    return _NC_CACHE[key]


# Results of the most recent kernel() call (exec_time_ns etc), for test.py.
LAST_RESULTS = None


def kernel(q, k, v, Wq, Wk, Wv):
    global LAST_RESULTS
    import os
    from concourse.bass_utils import run_bass_kernel_spmd

    q = np.asarray(q, dtype=np.float32)
    k = np.asarray(k, dtype=np.float32)
    v = np.asarray(v, dtype=np.float32)
    Wq = np.asarray(Wq, dtype=np.float32)
    Wk = np.asarray(Wk, dtype=np.float32)
    Wv = np.asarray(Wv, dtype=np.float32)

    nc = _get_compiled("full")

    in_maps = []
    for c in range(N_CORES):
        b, g = divmod(c, HPC)
        sl = slice(g * HD, (g + 1) * HD)
        in_maps.append({
            "q": np.ascontiguousarray(q[b]),
            "k": np.ascontiguousarray(k[b]),
            "v": np.ascontiguousarray(v[b]),
            "wq": np.ascontiguousarray(Wq[sl]),
            "wk": np.ascontiguousarray(Wk[sl]),
            "wv": np.ascontiguousarray(Wv[sl]),
        })

    res = run_bass_kernel_spmd(nc, in_maps, list(range(N_CORES)),
                               trace=bool(os.environ.get("MHA_TRACE")),
                               tmpdir=os.environ.get("MHA_TRACE_DIR") or None)
    LAST_RESULTS = res

    out = np.empty((B, S, D), dtype=np.float32)
    attn = np.empty((B, H, S, S), dtype=np.float32)
    for c in range(N_CORES):
        b, g = divmod(c, HPC)
        out[b, :, g * HD:(g + 1) * HD] = res.results[c]["out"]
        attn[b, g * HPC:(g + 1) * HPC] = res.results[c]["attn"]
    return out, attn


# revision 13
# speedup vs baseline: 1.0619x; 1.0619x over previous
"""Multi-head attention Bass/Tile kernel for Trainium2, 8-core SPMD.

Problem: B=2, S=2048, D=1024, H=16, DK=64 MHA forward returning
(out [B,S,D], attn [B,H,S,S]).

Sharding: core c handles batch b = c // 4 and head group g = c % 4
(heads 4g..4g+3).  Each core computes its 4 heads' attention entirely
locally; no collectives.  Host slices inputs / reassembles outputs.

Per-core pipeline (f32 data; matmul operands stored as float32r, which the
PE streams at full rate vs 1/4 rate for plain fp32):
  - transpose q,k,v tiles on PE (contraction dim must sit on partitions)
  - qhT,khT = W @ xT   (feature-major [256, S]);  vh = x @ WvT (token-major)
    with an appended ones-column -> AV matmul also yields softmax row-sums
  - per (head, 512-query chunk):
      scoresT[n,m] = khT.T-stationary @ qhT-moving   (PSUM)
      expT = exp(scoresT / 8)                        (ScalarE, PSUM->SBUF)
      outT' += vh'[ntile].T @ expT                   (PE accumulate, 65 rows)
      attn rows = transpose(expT) * recip(sums)      (PE transpose + VectorE)
"""

import numpy as np

import concourse.bass as bass
import concourse.mybir as mybir
from concourse import bacc
from concourse.tile import TileContext
from concourse.masks import make_identity

F32 = mybir.dt.float32
F32R = mybir.dt.float32r
F16 = mybir.dt.float16

B, S, D, H, DK = 2, 2048, 1024, 16, 64
P = 128
HPC = 4            # heads per core
HD = HPC * DK      # 256 head-dim columns per core
N_CORES = 8


def build_mha_kernel(seq=S, d_model=D, hpc=HPC, m_chunk=512, use_f32r=True):
    """Build the per-core Bass program. Returns the compiled Bacc object."""
    dk = DK
    hd = hpc * dk
    kt_n = d_model // P          # k-tiles over model dim (8)
    half = min(256, seq)         # prologue half-chunk (tokens)
    n_half = seq // half
    nch = seq // m_chunk         # query chunks per head
    ms_n = m_chunk // P          # 128-row subtiles per chunk
    nt_n = seq // P              # key tiles (16)
    dt_n = hd // P               # feature tiles of qhT/khT (2)

    AF = F16 if use_f32r else F32    # matmul operand storage dtype

    nc = bacc.Bacc("TRN2", target_bir_lowering=False, debug=False,
                   num_devices=N_CORES)

    q_d = nc.dram_tensor("q", [seq, d_model], F32, kind="ExternalInput").ap()
    k_d = nc.dram_tensor("k", [seq, d_model], F32, kind="ExternalInput").ap()
    v_d = nc.dram_tensor("v", [seq, d_model], F32, kind="ExternalInput").ap()
    wq_d = nc.dram_tensor("wq", [hd, d_model], F32, kind="ExternalInput").ap()
    wk_d = nc.dram_tensor("wk", [hd, d_model], F32, kind="ExternalInput").ap()
    wv_d = nc.dram_tensor("wv", [hd, d_model], F32, kind="ExternalInput").ap()
    attn_d = nc.dram_tensor("attn", [hpc, seq, seq], F32,
                            kind="ExternalOutput").ap()
    out_d = nc.dram_tensor("out", [seq, hd], F32, kind="ExternalOutput").ap()

    with TileContext(nc) as tc:
        with (
            tc.tile_pool(name="const", bufs=1) as const_pool,
            tc.tile_pool(name="wts", bufs=1) as wts_pool,
            tc.tile_pool(name="persist", bufs=1) as persist_pool,
            tc.tile_pool(name="xin", bufs=2) as xin_pool,
            tc.tile_pool(name="xt", bufs=2) as xt_pool,
            tc.tile_pool(name="expt", bufs=2 * nt_n + 2) as expt_pool,
            tc.tile_pool(name="arow", bufs=2) as arow_pool,
            tc.tile_pool(name="small", bufs=4) as small_pool,
            tc.tile_pool(name="outsb", bufs=2) as outsb_pool,
            tc.tile_pool(name="ps_a", bufs=2, space="PSUM") as ps_a,      # scores / proj
            tc.tile_pool(name="ps_t", bufs=2, space="PSUM") as ps_t,      # transposes
            tc.tile_pool(name="ps_av", bufs=2, space="PSUM") as ps_av,    # AV accum
            tc.tile_pool(name="ps_s", bufs=2, space="PSUM") as ps_s,      # small
        ):
            ident = const_pool.tile([P, P], F32)
            make_identity(nc, ident)
            ident_h = const_pool.tile([P, P], F16)
            nc.vector.tensor_copy(ident_h, ident)

            # ---- weights: load [hd, D] then transpose to wT[128k, kt, hd] ----
            wT = {}
            for name, w_dram in (("wq", wq_d), ("wk", wk_d), ("wv", wv_d)):
                w_sb = xin_pool.tile([P, dt_n, d_model], F32, tag="xload")
                nc.sync.dma_start(
                    out=w_sb,
                    in_=w_dram.rearrange("(a p) d -> p a d", p=P))
                wt_sb = wts_pool.tile([P, kt_n, hd], AF, tag=f"wt_{name}")
                for kt in range(kt_n):
                    pt = ps_t.tile([P, hd], F32, tag="t")
                    for a in range(dt_n):
                        nc.tensor.matmul(pt[:, a * P:(a + 1) * P],
                                         w_sb[:, a, kt * P:(kt + 1) * P],
                                         ident, is_transpose=True)
                    nc.vector.tensor_copy(wt_sb[:, kt, :], pt)
                wT[name] = wt_sb

            # ---- persistent activations ----
            qhT = persist_pool.tile([P, dt_n, seq], AF)   # [d-col, m]
            khT = persist_pool.tile([P, dt_n, seq], AF)
            vh = persist_pool.tile([P, nt_n, hpc, dk + 1], F16)  # + ones col
            ones_sb = const_pool.tile([P, nt_n], F32)
            nc.vector.memset(ones_sb, 1.0)
            for hh in range(hpc):  # ones column (f32 -> f32r rounding copy)
                nc.vector.tensor_copy(vh[:, :, hh, dk], ones_sb)

            # ---- transpose inputs + projections, streamed per half-chunk ----
            for name, x_dram in (("q", q_d), ("k", k_d), ("v", v_d)):
                for hc in range(n_half):
                    r0 = hc * half
                    sub_n = half // P  # 128-token subtiles in this half (2)
                    x_sb = xin_pool.tile([P, sub_n, d_model], F32, tag="xload")
                    nc.sync.dma_start(
                        out=x_sb,
                        in_=x_dram[r0:r0 + half, :].rearrange(
                            "(a p) d -> p a d", p=P))
                    xT = xt_pool.tile([P, kt_n, half], AF)
                    for kt in range(kt_n):
                        pt = ps_t.tile([P, half], F32, tag="t")
                        for a in range(sub_n):
                            nc.tensor.matmul(pt[:, a * P:(a + 1) * P],
                                             x_sb[:, a, kt * P:(kt + 1) * P],
                                             ident, is_transpose=True)
                        nc.scalar.copy(xT[:, kt, :], pt)  # ACT idle in prologue
                    if name in ("q", "k"):
                        dst = qhT if name == "q" else khT
                        wt_sb = wT["wq" if name == "q" else "wk"]
                        for dt in range(dt_n):
                            pp = ps_a.tile([P, half], F32, tag="a")
                            for kt in range(kt_n):
                                nc.tensor.matmul(
                                    pp,
                                    wt_sb[:, kt, dt * P:(dt + 1) * P],
                                    xT[:, kt, :],
                                    start=(kt == 0), stop=(kt == kt_n - 1))
                            nc.vector.tensor_copy(dst[:, dt, r0:r0 + half], pp)
                    else:
                        wt_sb = wT["wv"]
                        for a in range(sub_n):
                            nt = (r0 + a * P) // P
                            pp = ps_a.tile([P, hd], F32, tag="a")
                            for kt in range(kt_n):
                                nc.tensor.matmul(
                                    pp,
                                    xT[:, kt, a * P:(a + 1) * P],
                                    wt_sb[:, kt, :],
                                    start=(kt == 0), stop=(kt == kt_n - 1))
                            nc.vector.tensor_copy(
                                vh[:, nt, :, 0:dk],
                                pp.rearrange("p (h e) -> p h e", h=hpc))

            # ---- main loop: chunk outer, head inner ----
            scale = 1.0 / np.sqrt(np.float32(dk))
            for c in range(nch):
                m0 = c * m_chunk
                out_c = outsb_pool.tile([P, ms_n, hd], F32)
                for h in range(hpc):
                    dt_h, off_h = divmod(h * dk, P)

                    # scores + exp per key tile
                    expts = []
                    for nt in range(nt_n):
                        pscore = ps_a.tile([P, m_chunk], F32, tag="a")
                        nc.tensor.matmul(
                            pscore,
                            khT[off_h:off_h + dk, dt_h, nt * P:(nt + 1) * P],
                            qhT[off_h:off_h + dk, dt_h, m0:m0 + m_chunk])
                        et = expt_pool.tile([P, m_chunk], F16, tag="expt")
                        nc.scalar.activation(et, pscore,
                                             mybir.ActivationFunctionType.Exp,
                                             scale=float(scale))
                        expts.append(et)

                    # AV accumulation (row 64 = softmax denominator)
                    pav = ps_av.tile([dk + 1, m_chunk], F32, tag="av")
                    for nt in range(nt_n):
                        nc.tensor.matmul(
                            pav,
                            vh[:, nt, h, :],
                            expts[nt],
                            start=(nt == 0), stop=(nt == nt_n - 1))
                    av_sb = small_pool.tile([dk + 1, m_chunk], F32, tag="avsb")
                    nc.vector.tensor_copy(av_sb, pav)

                    for ms in range(ms_n):
                        # transpose [65, 128] -> [128, 65]: cols 0..63 are the
                        # unnormalized output rows, col 64 the softmax denom
                        po = ps_s.tile([P, dk + 1], F32, tag="s")
                        nc.tensor.matmul(po,
                                         av_sb[:, ms * P:(ms + 1) * P],
                                         ident[0:dk + 1, 0:dk + 1],
                                         is_transpose=True)
                        rcol = small_pool.tile([P, 1], F32, tag="rcolsb")
                        nc.vector.reciprocal(rcol, po[:, dk:dk + 1])

                        # attn row block [128 queries, seq keys]
                        arow = arow_pool.tile([P, seq], F32, tag="arow")
                        g4 = min(4, nt_n)
                        for ng in range(nt_n // g4):
                            pt = ps_t.tile([P, g4 * P], F16, tag="t")
                            for j in range(g4):
                                nt = ng * g4 + j
                                nc.tensor.matmul(
                                    pt[:, j * P:(j + 1) * P],
                                    expts[nt][:, ms * P:(ms + 1) * P],
                                    ident_h[:, 0:P], is_transpose=True)
                            nc.vector.tensor_scalar_mul(
                                arow[:, ng * g4 * P:(ng + 1) * g4 * P],
                                pt, rcol)
                        nc.sync.dma_start(
                            out=attn_d[h, m0 + ms * P:m0 + (ms + 1) * P, :],
                            in_=arow)

                        # output block [128 queries, dk]
                        nc.vector.tensor_scalar_mul(
                            out_c[:, ms, h * dk:(h + 1) * dk],
                            po[:, 0:dk], rcol)

                nc.sync.dma_start(
                    out=out_d[m0:m0 + m_chunk, :].rearrange(
                        "(a p) e -> p a e", p=P),
                    in_=out_c)

    nc.compile()
    return nc


_NC_CACHE = {}


def _get_compiled(key, **kw):
    if key not in _NC_CACHE:
        _NC_CACHE[key] = build_mha_kernel(**kw)
    return _NC_CACHE[key]


# Results of the most recent kernel() call (exec_time_ns etc), for test.py.
LAST_RESULTS = None


def kernel(q, k, v, Wq, Wk, Wv):
    global LAST_RESULTS
    import os
    from concourse.bass_utils import run_bass_kernel_spmd

    q = np.asarray(q, dtype=np.float32)
    k = np.asarray(k, dtype=np.float32)
    v = np.asarray(v, dtype=np.float32)
    Wq = np.asarray(Wq, dtype=np.float32)
    Wk = np.asarray(Wk, dtype=np.float32)
    Wv = np.asarray(Wv, dtype=np.float32)

    nc = _get_compiled("full")

    in_maps = []
    for c in range(N_CORES):
        b, g = divmod(c, HPC)
        sl = slice(g * HD, (g + 1) * HD)
        in_maps.append({
            "q": np.ascontiguousarray(q[b]),
            "k": np.ascontiguousarray(k[b]),
            "v": np.ascontiguousarray(v[b]),
            "wq": np.ascontiguousarray(Wq[sl]),
            "wk": np.ascontiguousarray(Wk[sl]),
            "wv": np.ascontiguousarray(Wv[sl]),
        })

    res = run_bass_kernel_spmd(nc, in_maps, list(range(N_CORES)),
                               trace=bool(os.environ.get("MHA_TRACE")),
                               tmpdir=os.environ.get("MHA_TRACE_DIR") or None)
    LAST_RESULTS = res

    out = np.empty((B, S, D), dtype=np.float32)
    attn = np.empty((B, H, S, S), dtype=np.float32)
    for c in range(N_CORES):
        b, g = divmod(c, HPC)
        out[b, :, g * HD:(g + 1) * HD] = res.results[c]["out"]
        attn[b, g * HPC:(g + 1) * HPC] = res.results[c]["attn"]
    return out, attn
